# revision 8
# baseline (speedup 1.0000x reference)
"""Trainium2 Bass kernel for a 4-layer dense transformer (CustomGPT1).

Full-input contract: kernel(**inputs) takes the unsharded inputs (B=16),
shards batch across 8 NeuronCores (2 examples/core, data-parallel; params
replicated), runs one SPMD Bass kernel, and gathers the full output.

v3 strategy per core/example:
- Residual stream RESIDENT in SBUF (bf16, in-place updates): no x/a DRAM
  round-trips at all; HBM traffic is weights + embeddings + output only.
- Scores h@h^T in fp8e4 with MatmulPerfMode.DoubleRow (2 k-tiles per
  instruction, 2x PE throughput). Softmax uses the provable bound
  max logit = sqrt(D) (LN rows have norm exactly sqrt(D) when ln_w==1,
  ln_b==0), folded as a constant shift into the exp bias -> et stays
  small; et kept bf16 (fp8 et loses too much precision: correlated
  rounding of equal-valued same-token keys).
- attnU (probs@h) and FFN in bf16 (precision-bound; fp8 fails 2e-2).
- FFN w2 pass computed with fg stationary / w2 moving -> s-major outputs,
  no output transposes, residual added straight from PSUM.
- Weights/embeddings pre-cast to bf16 on host; denominators via
  DVE-accumulated dacc + PE transpose + free-axis reduce.
- gpsimd (Pool) engine does psum->sbuf copy/casts (hT fp8, n2T bf16),
  keeping DVE for LN applies, dacc, and fused residual epilogues.
"""
import sys
sys.path.insert(0, "/opt/trn_rl_repo")
import math
import numpy as np
import ml_dtypes
import concourse.bass as bass
import concourse.mybir as mybir
import concourse.tile as tile
from concourse import bacc
from concourse.bass_utils import run_bass_kernel_spmd
from concourse.masks import make_identity

F32 = mybir.dt.float32
BF16 = mybir.dt.bfloat16
FP8 = mybir.dt.float8e4
I32 = mybir.dt.int32
AF = mybir.ActivationFunctionType
OP = mybir.AluOpType
AX = mybir.AxisListType
DR = mybir.MatmulPerfMode.DoubleRow

B, S, D, L, FF, V = 16, 2048, 512, 4, 2048, 33
NCORES, BL = 8, B // 8          # 2 examples per core
P = 128
SB = S // P                     # 16 s-blocks per example
DC = D // P                     # 4 d-chunks
FC = FF // P                    # 16 f-chunks
ASC = 512                       # attention s-chunk width
NASC = S // ASC                 # 4
ASB = ASC // P                  # 4 s-blocks per attention chunk
SCALE = 1.0 / math.sqrt(D)
SHIFT = math.sqrt(D)            # provable max logit in simple mode
EPS = 1e-5
VP = V + 1                      # psum-friendly padded vocab


def build(simple):
    nc = bacc.Bacc(None, target_bir_lowering=False)

    ids = nc.dram_tensor("ids", [BL, S], I32, kind="ExternalInput")
    aidx = nc.dram_tensor("aidx", [BL], I32, kind="ExternalInput")
    mask = nc.dram_tensor("mask", [BL, S], F32, kind="ExternalInput")
    tok_emb = nc.dram_tensor("tok_emb", [V, D], BF16, kind="ExternalInput")
    pos_emb = nc.dram_tensor("pos_emb", [S, D], BF16, kind="ExternalInput")
    attr_emb = nc.dram_tensor("attr_emb", [608, D], BF16, kind="ExternalInput")
    lnw = nc.dram_tensor("lnw", [L, D], F32, kind="ExternalInput")
    lnb = nc.dram_tensor("lnb", [L, D], F32, kind="ExternalInput")
    w1 = nc.dram_tensor("w1", [L, D, FF], BF16, kind="ExternalInput")
    b1 = nc.dram_tensor("b1", [L, FF], F32, kind="ExternalInput")
    w2 = nc.dram_tensor("w2", [L, FF, D], BF16, kind="ExternalInput")
    b2 = nc.dram_tensor("b2", [L, D], F32, kind="ExternalInput")
    out_w = nc.dram_tensor("out_w", [D, V], BF16, kind="ExternalInput")
    out_b = nc.dram_tensor("out_b", [V], F32, kind="ExternalInput")
    out = nc.dram_tensor("out", [BL, S, V], F32, kind="ExternalOutput")

    def bcast_row(handle, offset, n):
        # [n]-vector at element `offset`, replicated across all 128 partitions
        return bass.AP(tensor=handle.ap().tensor, offset=offset, ap=[[0, P], [1, n]])

    with tile.TileContext(nc) as tc:
        with tc.tile_pool(name="cst", bufs=1) as cst, \
             tc.tile_pool(name="parw", bufs=2) as parw, \
             tc.tile_pool(name="w1p", bufs=2) as w1p, \
             tc.tile_pool(name="w2p", bufs=1) as w2p, \
             tc.tile_pool(name="hn", bufs=2) as hn, \
             tc.tile_pool(name="trq", bufs=2) as trq, \
             tc.tile_pool(name="trn", bufs=2) as trn, \
             tc.tile_pool(name="tmp", bufs=2) as tmp, \
             tc.tile_pool(name="sml", bufs=6) as sml, \
             tc.tile_pool(name="psA", bufs=4, space="PSUM") as psA, \
             tc.tile_pool(name="psS", bufs=2, space="PSUM") as psS, \
             tc.tile_pool(name="psT", bufs=2, space="PSUM") as psT:

            # ---------------- constants / small parameters ----------------
            ident_f = cst.tile([P, P], F32, tag="identf")
            make_identity(nc, ident_f)
            ident_b = cst.tile([P, P], BF16, tag="identb")
            nc.vector.tensor_copy(ident_b, ident_f)
            eps_t = cst.tile([P, 1], F32, tag="eps")
            nc.vector.memset(eps_t, EPS)

            # per-partition index column (for transposed one-hot embedding)
            iotap_i = cst.tile([P, 1], I32, tag="iotapi")
            nc.gpsimd.iota(iotap_i, pattern=[[1, 1]], base=0, channel_multiplier=1)
            iotap = cst.tile([P, 1], F32, tag="iotap")
            nc.vector.tensor_copy(iotap, iotap_i)

            # token-embedding table resident: [v, d] with v on partitions
            tokT = cst.tile([P, D], BF16, tag="tokt")
            nc.sync.dma_start(out=tokT[:V, :], in_=tok_emb.ap())

            outb_b = cst.tile([P, V], F32, tag="outb")
            nc.sync.dma_start(out=outb_b, in_=bcast_row(out_b, 0, V))
            outw_sb = cst.tile([P, DC, VP], BF16, tag="outw")
            nc.vector.memset(outw_sb, 0.0)
            nc.sync.dma_start(out=outw_sb[:, :, :V], in_=out_w.ap().rearrange("(do p) v -> p do v", p=P))

            # exp bias per example: (m*1e9 - (1e9 + shift)), layout [t_in=128, tc=16]
            shift = SHIFT if simple else 0.0
            maskb = []
            for b in range(BL):
                ml = sml.tile([P, SB], F32, tag=f"mload{b}")
                nc.sync.dma_start(out=ml, in_=mask.ap()[b].rearrange("(tc p) -> p tc", p=P))
                mb = cst.tile([P, SB], F32, tag=f"maskb{b}")
                nc.vector.tensor_scalar(out=mb, in0=ml, scalar1=1e9, scalar2=1e9 + shift,
                                        op0=OP.mult, op1=OP.subtract)
                maskb.append(mb)

            # per-example token table with the attribute row folded in
            tokTb = []
            for b in range(BL):
                ai = sml.tile([P, 1], I32, tag="aidx", name=f"ai{b}")
                nc.sync.dma_start(out=ai, in_=bass.AP(tensor=aidx.ap().tensor, offset=b, ap=[[0, P], [1, 1]]))
                attrv = sml.tile([P, D], BF16, tag=f"attrv{b}")
                nc.gpsimd.indirect_dma_start(
                    out=attrv[:, :], out_offset=None, in_=attr_emb[:, :],
                    in_offset=bass.IndirectOffsetOnAxis(ap=ai[:, :1], axis=0))
                tb = cst.tile([P, D], BF16, tag=f"tokTb{b}")
                nc.vector.tensor_tensor(out=tb[:V, :], in0=tokT[:V, :], in1=attrv[:V, :], op=OP.add)
                tokTb.append(tb)

            # residual streams, persistent in SBUF
            xs = [cst.tile([P, SB, D], BF16, tag=f"x{b}", name=f"x{b}") for b in range(BL)]

            def stats_into(mvall, xt, sb):
                st = sml.tile([P, 6], F32, tag="st", name="st")
                nc.vector.bn_stats(st, xt)
                nc.vector.bn_aggr(mvall[:, sb, :], st)

            def finish_stats(mvall):
                rstdall = sml.tile([P, SB], F32, tag="rstdall", bufs=4, name="rstdall")
                nc.scalar.activation(rstdall, mvall[:, :, 1:2], AF.Sqrt, bias=eps_t, scale=1.0)
                nc.vector.reciprocal(rstdall, rstdall)
                return rstdall

            def ln_apply(xt, mvall, rstdall, sb, lnw_b, lnb_b, out_slice):
                nc.vector.tensor_scalar(out=out_slice, in0=xt, scalar1=mvall[:, sb, 0:1],
                                        scalar2=rstdall[:, sb:sb + 1],
                                        op0=OP.subtract, op1=OP.mult)
                if not simple:
                    nc.vector.tensor_tensor(out=out_slice, in0=out_slice, in1=lnw_b, op=OP.mult)
                    nc.vector.tensor_tensor(out=out_slice, in0=out_slice, in1=lnb_b, op=OP.add)

            def transpose_to(dst, src_tile, sb, dt):
                """PE-transpose a [s128, D] tile into dst[:, :, sb*128:(sb+1)*128]
                (d-major), via psum; psum->sbuf copy (+cast) on gpsimd."""
                r0 = sb * P
                pt = psT.tile([P, D], BF16, tag="pt", name="pt")
                for dc in range(DC):
                    nc.tensor.transpose(pt[:, dc * P:(dc + 1) * P], src_tile[:, dc * P:(dc + 1) * P], ident_b)
                nc.vector.tensor_copy(dst[:, :, r0:r0 + P], pt.rearrange("p (dc q) -> p dc q", q=P))

            # ---------------- embeddings (layer-0 input) ----------------
            mv1_next = {}
            for b in range(BL):
                mv1_next[b] = sml.tile([P, SB, 2], F32, tag=f"mv1_{b}", bufs=2, name=f"mv1e{b}")
            for sb in range(SB):
                for b in range(BL):
                    r0 = sb * P
                    idr_i = sml.tile([P, P], I32, tag="idri", bufs=3, name="idr_i")
                    nc.sync.dma_start(out=idr_i, in_=bass.AP(tensor=ids.ap().tensor,
                                                             offset=b * S + r0, ap=[[0, P], [1, P]]))
                    idr = sml.tile([P, P], F32, tag="idr", bufs=3, name="idr")
                    nc.vector.tensor_copy(idr, idr_i)
                    ohT = tmp.tile([P, P], BF16, tag="ohTs", bufs=3, name="ohT")
                    nc.vector.tensor_scalar(out=ohT[:V, :], in0=idr[:V, :], scalar1=iotap[:V, :1],
                                            scalar2=None, op0=OP.is_equal)
                    tokv = psS.tile([P, D], F32, tag="ps", name="tokv")
                    nc.tensor.matmul(tokv, ohT[:V, :], tokTb[b][:V, :], start=True, stop=True)
                    pe = tmp.tile([P, D], BF16, tag="pos", bufs=3, name="pe")
                    nc.sync.dma_start(out=pe, in_=pos_emb.ap()[r0:r0 + P, :])
                    xsl = xs[b][:, sb, :]
                    nc.vector.scalar_tensor_tensor(out=xsl, in0=tokv, scalar=1.0, in1=pe,
                                                   op0=OP.mult, op1=OP.add)
                    stats_into(mv1_next[b], xsl, sb)

            # ---------------- layers ----------------
            for l in range(L):
                w1sb = w1p.tile([P, DC, FF], BF16, tag="w1")
                nc.sync.dma_start(out=w1sb, in_=w1.ap()[l].rearrange("(do p) f -> p do f", p=P))
                w2sb = w2p.tile([P, FC, D], BF16, tag="w2")
                nc.sync.dma_start(out=w2sb, in_=w2.ap()[l].rearrange("(fc p) d -> p fc d", p=P))
                if not simple:
                    lnw_b = parw.tile([P, D], F32, tag="lnw")
                    nc.sync.dma_start(out=lnw_b, in_=bcast_row(lnw, l * D, D))
                    lnb_b = parw.tile([P, D], F32, tag="lnb")
                    nc.sync.dma_start(out=lnb_b, in_=bcast_row(lnb, l * D, D))
                    b2sb = parw.tile([P, D], F32, tag="b2")
                    nc.sync.dma_start(out=b2sb, in_=bcast_row(b2, l * D, D))
                else:
                    lnw_b = lnb_b = b2sb = None
                b1sb = parw.tile([P, FC], F32, tag="b1")
                nc.sync.dma_start(out=b1sb, in_=b1.ap()[l].rearrange("(fc p) -> p fc", p=P))

                last = (l == L - 1)
                mvall1 = {b: mv1_next[b] for b in range(BL)}
                rstd1 = {}
                h_nat = {}
                hT = {}
                # ---- LN1 both examples: h_nat (s-major bf16) + hT (d-major fp8)
                for b in range(BL):
                    h_nat[b] = hn.tile([P, SB, D], BF16, tag="h", name=f"h{b}")
                    hT[b] = trq.tile([P, DC, S], FP8, tag="hT", name=f"hT{b}")
                    rstd1[b] = finish_stats(mvall1[b])
                    for sb in range(SB):
                        hsl = h_nat[b][:, sb, :]
                        ln_apply(xs[b][:, sb, :], mvall1[b], rstd1[b], sb, lnw_b, lnb_b, hsl)
                        transpose_to(hT[b], hsl, sb, FP8)

                mvall2 = {}
                for b in range(BL):
                    # ---- attention: fp8-DR scores -> exp(bf16) -> bf16 attnU
                    mvall2[b] = sml.tile([P, SB, 2], F32, tag="mv2", bufs=2, name=f"mv2{b}")
                    for sc in range(NASC):
                        c0 = sc * ASC
                        pa = [psA.tile([P, D], F32, tag="pa", name=f"pa{_h}") for _h in range(ASB)]
                        dacc = tmp.tile([P, ASC], F32, tag="dacc", name="dacc")
                        for tc_i in range(SB):
                            ps_sc = psS.tile([P, ASC], F32, tag="ps", name="ps_sc")
                            for kg in range(0, DC, 2):
                                nc.tensor.matmul(ps_sc, hT[b][:, kg:kg + 2, tc_i * P:(tc_i + 1) * P],
                                                 hT[b][:, kg:kg + 2, c0:c0 + ASC],
                                                 start=(kg == 0), stop=(kg == DC - 2),
                                                 perf_mode=DR)
                            et = tmp.tile([P, ASC], BF16, tag="expt", bufs=4, name="et")
                            nc.scalar.activation(et, ps_sc, AF.Exp,
                                                 bias=maskb[b][:, tc_i:tc_i + 1], scale=SCALE)
                            if tc_i == 0:
                                nc.vector.tensor_copy(dacc, et)
                            else:
                                nc.vector.tensor_tensor(out=dacc, in0=dacc, in1=et, op=OP.add)
                            for hf in range(ASB):
                                nc.tensor.matmul(pa[hf][:, :], et[:, hf * P:(hf + 1) * P],
                                                 h_nat[b][:, tc_i, :],
                                                 start=(tc_i == 0), stop=(tc_i == SB - 1))
                        # denominators: PE-transpose dacc -> [s, t] blocks, reduce t,
                        # reciprocal -> per-partition scalars
                        daccb = tmp.tile([P, ASC], BF16, tag="daccb", name="daccb")
                        nc.vector.tensor_copy(daccb, dacc)
                        den_t = psT.tile([P, ASC], BF16, tag="pt", name="den_t")
                        for hf in range(ASB):
                            nc.tensor.transpose(den_t[:, hf * P:(hf + 1) * P],
                                                daccb[:, hf * P:(hf + 1) * P], ident_b)
                        dsb = sml.tile([P, ASB], F32, tag="dsb", name="dsb")
                        nc.vector.tensor_reduce(out=dsb,
                                                in_=den_t.rearrange("p (hf q) -> p hf q", q=P),
                                                op=OP.add, axis=AX.X)
                        drec = sml.tile([P, ASB], F32, tag="drec", name="drec")
                        nc.vector.reciprocal(drec, dsb)
                        for hf in range(ASB):
                            sb_i = sc * ASB + hf
                            xsl = xs[b][:, sb_i, :]
                            nc.vector.scalar_tensor_tensor(out=xsl, in0=pa[hf],
                                                           scalar=drec[:, hf:hf + 1], in1=xsl,
                                                           op0=OP.mult, op1=OP.add)
                            stats_into(mvall2[b], xsl, sb_i)

                    # ---- LN2 -> n2T (d-major bf16)
                    n2T = trn.tile([P, DC, S], BF16, tag="n2T", name="n2T")
                    rstd2 = finish_stats(mvall2[b])
                    for sb in range(SB):
                        n2 = tmp.tile([P, D], BF16, tag="n2", bufs=3, name="n2")
                        ln_apply(xs[b][:, sb, :], mvall2[b], rstd2, sb, lnw_b, lnb_b, n2[:, :])
                        transpose_to(n2T, n2, sb, BF16)

                    # ---- FFN (fg stationary / w2 moving -> s-major outputs)
                    if not last:
                        mv1_next[b] = sml.tile([P, SB, 2], F32, tag=f"mv1_{b}", bufs=2,
                                               name=f"mv1n{b}")
                    for fs in range(NASC):
                        c0 = fs * ASC
                        p2 = [psA.tile([P, D], F32, tag="pa", name=f"p2_{_s}") for _s in range(ASB)]
                        for fc in range(FC):
                            pf = psS.tile([P, ASC], F32, tag="ps", name="pf")
                            for do in range(DC):
                                nc.tensor.matmul(pf, w1sb[:, do, fc * P:(fc + 1) * P],
                                                 n2T[:, do, c0:c0 + ASC],
                                                 start=(do == 0), stop=(do == DC - 1))
                            fg = tmp.tile([P, ASC], BF16, tag="ffg", bufs=3, name="fg")
                            nc.scalar.activation(fg, pf, AF.Gelu, bias=b1sb[:, fc:fc + 1], scale=1.0)
                            for ss in range(ASB):
                                nc.tensor.matmul(p2[ss], fg[:, ss * P:(ss + 1) * P],
                                                 w2sb[:, fc, :],
                                                 start=(fc == 0), stop=(fc == FC - 1))
                        for ss in range(ASB):
                            sb_i = fs * ASB + ss
                            xsl = xs[b][:, sb_i, :]
                            if not simple:
                                nc.vector.tensor_tensor(out=p2[ss], in0=p2[ss], in1=b2sb, op=OP.add)
                            if not last:
                                nc.vector.tensor_tensor(out=xsl, in0=p2[ss], in1=xsl, op=OP.add)
                                stats_into(mv1_next[b], xsl, sb_i)
                            else:
                                # final projection fused into the epilogue
                                r0 = sb_i * P
                                xnr = tmp.tile([P, D], BF16, tag="xn", name="xnr")
                                nc.vector.tensor_tensor(out=xnr, in0=p2[ss], in1=xsl, op=OP.add)
                                pt = psT.tile([P, D], BF16, tag="pt", name="ptx")
                                for dc in range(DC):
                                    nc.tensor.transpose(pt[:, dc * P:(dc + 1) * P],
                                                        xnr[:, dc * P:(dc + 1) * P], ident_b)
                                xtsb = tmp.tile([P, DC, P], BF16, tag="xtsb", name="xtsb")
                                nc.vector.tensor_copy(xtsb, pt.rearrange("p (dc q) -> p dc q", q=P))
                                po_t = psS.tile([P, ASC], F32, tag="ps", name="po_t")
                                po = po_t[:, :VP]
                                for do in range(DC):
                                    nc.tensor.matmul(po, xtsb[:, do, :], outw_sb[:, do, :],
                                                     start=(do == 0), stop=(do == DC - 1))
                                ot = tmp.tile([P, V], F32, tag="ot", name="ot")
                                nc.vector.tensor_tensor(out=ot, in0=po[:, :V], in1=outb_b, op=OP.add)
                                nc.scalar.dma_start(out=out[b, r0:r0 + P, :], in_=ot)

    nc.compile()
    return nc


_NC = {}


def _get_nc(simple=True):
    if simple not in _NC:
        _NC[simple] = build(simple)
    return _NC[simple]


def _is_simple(inputs):
    return (np.all(np.asarray(inputs["ln_w"]) == 1.0)
            and np.all(np.asarray(inputs["ln_b"]) == 0.0)
            and np.all(np.asarray(inputs["b2"]) == 0.0))


def make_in_maps(inputs):
    f = lambda a: np.ascontiguousarray(np.asarray(a, dtype=np.float32))
    h = lambda a: np.ascontiguousarray(np.asarray(a, dtype=np.float32).astype(ml_dtypes.bfloat16))
    i = lambda a: np.ascontiguousarray(np.asarray(a, dtype=np.int32))
    shared = {
        "tok_emb": h(inputs["tok_emb"]), "pos_emb": h(inputs["pos_emb"]),
        "attr_emb": h(inputs["attr_emb"]),
        "lnw": f(inputs["ln_w"]), "lnb": f(inputs["ln_b"]),
        "w1": h(inputs["w1"]), "b1": f(inputs["b1"]),
        "w2": h(inputs["w2"]), "b2": f(inputs["b2"]),
        "out_w": h(inputs["out_w"]), "out_b": f(inputs["out_b"]),
    }
    in_maps = []
    for c in range(NCORES):
        sl = slice(BL * c, BL * (c + 1))
        m = dict(shared)
        m["ids"] = i(inputs["input_ids"][sl])
        m["aidx"] = i(inputs["combined_indices"][sl])
        m["mask"] = f(inputs["attention_mask"][sl])
        in_maps.append(m)
    return in_maps


def kernel(**inputs):
    res = run_bass_kernel_spmd(_get_nc(_is_simple(inputs)), make_in_maps(inputs),
                               core_ids=list(range(NCORES)))
    return np.concatenate([r["out"] for r in res.results], axis=0)
